# revision 10
# baseline (speedup 1.0000x reference)
"""Distillation-trainer loss kernel for Trainium2 (8 NeuronCores).

Computes  loss = mean((attn(q,k,v) - attn(q,ck,cv))**2)  for
q:[1,8,1024,128], k/v:[1,8,8192,128], ck/cv:[1,8,1024,128] fp32.

Sharding: one kv-head per core (h axis, 8 heads / 8 cores). Each core
computes its head's squared-error partial sums; the host adds the 8
partials and divides by the element count (the "all-reduce" of the
scalar loss).

Host-side prep (part of sharding): per head, ship bf16 operands in the
exact SBUF layouts the PE needs — kT/qT/ckT pre-transposed to [d, n],
v/cv pre-swizzled to [128p, t, d] with a ones column appended (the
denominator trick). This removes all on-device transposes/casts and
halves DMA bytes. Input DMAs are spread across engine queues so the
kT and vb streams transfer in parallel.

Per-core algorithm (head h), per 256-wide q-chunk:
  - scoresT[n, q] = kT-tile.T @ qT-chunk on PE in bf16 (fp32 PSUM).
    Scores grouped 4 n-tiles (2 PSUM banks) x 3 buffers so TWO exp
    engines run concurrently on different groups:
      ACT:  expT = Exp(scoresT * 1/sqrt(d)) -> bf16 (even groups).
      DVE:  Schraudolph in bf16 (odd groups): i16 = rint(s*A16 + B16)
            written through a bitcast into the bf16 tile; the int16 bit
            pattern IS the bf16 exp approximation (~2% multiplicative
            noise, zero-mean through softmax; loss rel-err ~3e-4).
  - PV emission lags the QK groups by 2 so exp latency (~1.2-1.5us) is
    hidden behind ~1.8us of PE work: stationary = expT chunk
    [128n, 128q], moving = v' [128n, 129]; PSUM accumulates z' | S.
  - ACT copies z'|S PSUM->SBUF right after the PV flush (frees the
    accumulation banks for the next attend with no DVE involvement);
    the DVE normalize/MSE math on those copies is deferred and
    interleaved into the NEXT attend's group loop, keeping the qc
    boundary free of serialized vector work:
      zcomp[qt] = z'c * 1/Sc   (compressed, via reciprocal + mul)
      acc[qt]  += sum((z'*invS - zcomp[qt])^2)  (two fused
                  scalar_tensor_tensor ops, accum_out row sums)
  - Compressed (NC=1024) and teacher (N=8192) attends interleave per
    q-chunk so the kT/vb DMA stream hides behind early compute.
"""

import numpy as np

import concourse.bass as bass
import concourse.mybir as mybir
import concourse.tile as tile
from concourse import bacc
from concourse.bass_utils import run_bass_kernel_spmd

F32 = mybir.dt.float32
BF16 = mybir.dt.bfloat16
I16 = mybir.dt.int16
AF = mybir.ActivationFunctionType
ALU = mybir.AluOpType

B, H, Q, N, NC, D = 1, 8, 1024, 8192, 1024, 128
N_CORES = 8
SCALE = 1.0 / float(np.sqrt(D))

QC = 256                   # q chunk width for the scores moving operand
N_QC = Q // QC             # 4
GT = 4                     # n-tiles per PSUM scores group (2 banks)
NT = N // 128              # 64 teacher n-tiles
NCT = NC // 128            # 8 compressed n-tiles
PV_LAG = 2                 # groups of PV emission lag behind QK

# Schraudolph-to-bf16 constants: exp(s*SCALE) ~= bf16_bits(rint(s*A16+B16)).
# HW DVE converts fp32->int16 with round-to-nearest (measured).
LN2 = float(np.log(2.0))
A16 = float(128.0 / LN2 * SCALE)
B16 = float(127 * 128 - 8)          # b_adj=8 minimizes softmax-weight bias


def teacher_on_dve(qc, gi):
    return gi % 2 == 1


def comp_on_dve(qc, gi):
    return gi == 1


def _emit(nc: bass.Bass, tc: tile.TileContext, qTh, kTh, vbh, ckTh, cvbh, out_dram):
    ctxs = []

    def pool(**kw):
        p = tc.tile_pool(**kw)
        ctxs.append(p)
        return p.__enter__()

    pconst = pool(name="pconst", bufs=1)
    pex = pool(name="pex", bufs=3)
    psmall = pool(name="psmall", bufs=8)
    psc = pool(name="psc", bufs=3, space="PSUM")
    pz = pool(name="pz", bufs=1, space="PSUM")

    # ---- persistent SBUF tensors ----
    kT = pconst.tile([128, NT, 128], BF16, tag="kT")        # [d, t, n]
    vb = pconst.tile([128, NT, 129], BF16, tag="vb")        # [p, t, d+1]
    qT = pconst.tile([128, Q], BF16, tag="qT")              # [d, q]
    ckT = pconst.tile([128, NCT, 128], BF16, tag="ckT")
    cvb = pconst.tile([128, NCT, 129], BF16, tag="cvb")
    zcomp = pconst.tile([128, Q // 128, 128], F32, tag="zcomp")  # [q, qt, d]
    accq = pconst.tile([128, Q // 128], F32, tag="accq")

    # Warm the ACT exp table immediately so the ~2.7us ACT_TABLE_LOAD is
    # off the first real exp's critical path.
    warm = psmall.tile([128, 1], F32, tag="warm")
    nc.gpsimd.memset(warm[:], 0.0)
    warm2 = psmall.tile([128, 1], F32, tag="warm2")
    nc.scalar.activation(warm2[:], warm[:], AF.Exp)

    # Warm the PE HAM clock gate during the input-DMA lead: ~3us of dummy
    # matmuls trips the activity monitor to K=8/8 (2.4 GHz) before the
    # first real matmul instead of ~8us into the compressed phase.
    wb = psmall.tile([128, 64], BF16, tag="wb")
    nc.gpsimd.memset(wb[:], 0.0)
    wps = psc.tile([128, GT, QC], F32, tag="sp")
    for _ in range(56):
        nc.tensor.matmul(wps[0:64, 0, 0:64], wb[:], wb[:],
                         start=True, stop=True)

    # ---- input DMAs (pre-transposed/swizzled bf16) spread over queues ----
    KCH = 4
    kt_per = NT // KCH

    def kchunk(c):
        return (kT[:, c * kt_per:(c + 1) * kt_per, :],
                kTh[:, c * kt_per * 128:(c + 1) * kt_per * 128]
                .rearrange("p (t n) -> p t n", t=kt_per))

    def vchunk(c):
        return (vb[:, c * kt_per:(c + 1) * kt_per, :],
                vbh[:, c * kt_per * 129:(c + 1) * kt_per * 129]
                .rearrange("p (t n) -> p t n", t=kt_per))

    nc.scalar.dma_start(out=ckT[:],
                        in_=ckTh[:, :].rearrange("p (t n) -> p t n", t=NCT))
    nc.sync.dma_start(out=qT[:, 0:QC], in_=qTh[:, 0:QC])
    nc.sync.dma_start(out=cvb[:],
                      in_=cvbh[:, :].rearrange("p (t n) -> p t n", t=NCT))
    # two parallel streams, ordered by first-use time; qT tail is not
    # needed until the second q-chunk so it goes last.
    for o, i in (kchunk(0), kchunk(1), vchunk(1), vchunk(3)):
        nc.sync.dma_start(out=o, in_=i)
    nc.sync.dma_start(out=qT[:, QC:Q], in_=qTh[:, QC:Q])
    for o, i in (kchunk(2), kchunk(3), vchunk(0), vchunk(2)):
        nc.gpsimd.dma_start(out=o, in_=i)

    # ---- attention + softmax-PV for one q-chunk of 256 ----
    def attend(keysT, vals, n_tiles, qc, on_dve, hooks=None):
        """Returns (za, zb) PSUM tiles [128, 129] = [z' | S] per q-half."""
        za = pz.tile([128, 129], F32, tag="za")
        zb = pz.tile([128, 129], F32, tag="zb")
        qs = qT[:, qc * QC:(qc + 1) * QC]

        def emit_pv(ex, t0, gn):
            for j in range(gn):
                t = t0 + j
                st = dict(start=(t == 0), stop=(t == n_tiles - 1))
                for c0, zp in ((0, za), (128, zb)):
                    nc.tensor.matmul(zp[:], ex[:, j, c0:c0 + 128],
                                     vals[:, t, :], **st)

        n_groups = n_tiles // GT
        pending = []
        for gi in range(n_groups):
            t0 = gi * GT
            sp = psc.tile([128, GT, QC], F32, tag="sp")
            for j in range(GT):
                nc.tensor.matmul(sp[:, j, :], keysT[:, t0 + j, :], qs,
                                 start=True, stop=True)
            if len(pending) >= PV_LAG:
                emit_pv(*pending.pop(0))
            ex = pex.tile([128, GT, QC], BF16, tag="ex")
            if on_dve(qc, gi):
                nc.vector.tensor_scalar(ex[:].bitcast(I16), sp[:], A16, B16,
                                        op0=ALU.mult, op1=ALU.add)
            else:
                nc.scalar.activation(ex[:], sp[:], AF.Exp, scale=SCALE)
            if hooks and gi in hooks:
                hooks[gi]()
            pending.append((ex, t0, GT))
        for p in pending:
            emit_pv(*p)
        return za, zb

    def act_drain(zp_pair):
        """ACT copies z'|S PSUM->SBUF, freeing the accumulation banks."""
        out = []
        for zp in zp_pair:
            zs = psmall.tile([128, 129], F32, tag="zs")
            nc.scalar.copy(zs[:], zp[:])
            out.append(zs)
        return out

    def zcomp_normalize(czs, qt):
        def run():
            inv = psmall.tile([128, 1], F32, tag="cinv")
            nc.vector.reciprocal(inv[:], czs[:, 128:129])
            nc.vector.tensor_scalar_mul(zcomp[:, qt, :], czs[:, 0:128], inv[:])
        return run

    def mse(zs, qt):
        def run():
            inv = psmall.tile([128, 1], F32, tag="inv")
            nc.vector.reciprocal(inv[:], zs[:, 128:129])
            d = psmall.tile([128, 128], F32, tag="d")
            nc.vector.scalar_tensor_tensor(d[:], zs[:, 0:128], inv[:],
                                           zcomp[:, qt, :],
                                           op0=ALU.mult, op1=ALU.subtract)
            d2 = psmall.tile([128, 128], F32, tag="d2")
            nc.vector.scalar_tensor_tensor(d2[:], d[:], 1.0, d[:],
                                           op0=ALU.mult, op1=ALU.mult,
                                           accum_out=accq[:, qt:qt + 1])
        return run

    # Interleaved phases per q-chunk; the DVE normalize/MSE of earlier
    # attends runs inside the next teacher attend's group loop.
    prev_mse = []
    for qc in range(N_QC):
        za, zb = attend(ckT, cvb, NCT, qc, comp_on_dve)
        czs = act_drain((za, zb))

        hooks = {2: zcomp_normalize(czs[0], qc * 2),
                 3: zcomp_normalize(czs[1], qc * 2 + 1)}
        for i, fn in enumerate(prev_mse):
            hooks[5 + 3 * i] = fn
        za, zb = attend(kT, vb, NT, qc, teacher_on_dve, hooks)
        zs = act_drain((za, zb))
        prev_mse = [mse(zs[0], qc * 2), mse(zs[1], qc * 2 + 1)]

    for fn in prev_mse:
        fn()

    nc.sync.dma_start(out=out_dram[:], in_=accq[:])

    for p in reversed(ctxs):
        p.__exit__(None, None, None)


_NC_CACHE = None


def build_nc():
    global _NC_CACHE
    if _NC_CACHE is not None:
        return _NC_CACHE
    nc = bacc.Bacc()
    qTh = nc.declare_dram_parameter("qT", [128, Q], BF16, isOutput=False)
    kTh = nc.declare_dram_parameter("kT", [128, N], BF16, isOutput=False)
    vbh = nc.declare_dram_parameter("vb", [128, NT * 129], BF16, isOutput=False)
    ckTh = nc.declare_dram_parameter("ckT", [128, NC], BF16, isOutput=False)
    cvbh = nc.declare_dram_parameter("cvb", [128, NCT * 129], BF16, isOutput=False)
    out = nc.declare_dram_parameter("loss_sums", [128, Q // 128], F32, isOutput=True)
    with tile.TileContext(nc) as tc:
        _emit(nc, tc, qTh, kTh, vbh, ckTh, cvbh, out)
    nc.compile()
    _NC_CACHE = nc
    return nc


NPBF16 = mybir.dt.np(BF16)


def _prep_head(qh, kh, vh, ckh, cvh):
    """Host-side shard prep: transpose/swizzle/cast one head's operands."""
    def swizzle_v(v):              # [n, d] -> [128p, t, d+1] with ones col
        t = v.shape[0] // 128
        vs = v.reshape(t, 128, D).transpose(1, 0, 2)
        out = np.empty((128, t, D + 1), dtype=NPBF16)
        out[:, :, 0:D] = vs.astype(NPBF16)
        out[:, :, D] = np.asarray(1.0, dtype=NPBF16)
        return out.reshape(128, t * (D + 1))

    return {
        "qT": np.ascontiguousarray(qh.T).astype(NPBF16),
        "kT": np.ascontiguousarray(kh.T).astype(NPBF16),
        "vb": swizzle_v(vh),
        "ckT": np.ascontiguousarray(ckh.T).astype(NPBF16),
        "cvb": swizzle_v(cvh),
    }


def make_in_maps(queries, keys, values, c_keys, c_values):
    in_maps = []
    for h in range(N_CORES):
        in_maps.append(_prep_head(
            np.asarray(queries[0, h], dtype=np.float32),
            np.asarray(keys[0, h], dtype=np.float32),
            np.asarray(values[0, h], dtype=np.float32),
            np.asarray(c_keys[0, h], dtype=np.float32),
            np.asarray(c_values[0, h], dtype=np.float32),
        ))
    return in_maps


def run_cores(in_maps, trace=False, **kw):
    nc = build_nc()
    return run_bass_kernel_spmd(nc, in_maps, list(range(N_CORES)),
                                trace=trace, **kw)


def kernel(queries, keys, values, c_keys, c_values):
    res = run_cores(make_in_maps(queries, keys, values, c_keys, c_values))
    total = sum(float(r["loss_sums"].astype(np.float64).sum())
                for r in res.results)
    loss = total / float(B * H * Q * D)
    return np.asarray(loss, dtype=np.float32)


# revision 11
# speedup vs baseline: 1.0024x; 1.0024x over previous
"""Distillation-trainer loss kernel for Trainium2 (8 NeuronCores).

Computes  loss = mean((attn(q,k,v) - attn(q,ck,cv))**2)  for
q:[1,8,1024,128], k/v:[1,8,8192,128], ck/cv:[1,8,1024,128] fp32.

Sharding: one kv-head per core (h axis, 8 heads / 8 cores). Each core
computes its head's squared-error partial sums; the host adds the 8
partials and divides by the element count (the "all-reduce" of the
scalar loss).

Host-side prep (part of sharding): per head, ship bf16 operands in the
exact SBUF layouts the PE needs — kT/qT/ckT pre-transposed to [d, n],
v/cv pre-swizzled to [128p, t, d] with a ones column appended (the
denominator trick). This removes all on-device transposes/casts and
halves DMA bytes. Input DMAs are spread across engine queues so the
kT and vb streams transfer in parallel.

Per-core algorithm (head h), per 256-wide q-chunk:
  - scoresT[n, q] = kT-tile.T @ qT-chunk on PE in bf16 (fp32 PSUM).
    Scores grouped 4 n-tiles (2 PSUM banks) x 3 buffers so TWO exp
    engines run concurrently on different groups:
      ACT:  expT = Exp(scoresT * 1/sqrt(d)) -> bf16 (even groups).
      DVE:  Schraudolph in bf16 (odd groups): i16 = rint(s*A16 + B16)
            written through a bitcast into the bf16 tile; the int16 bit
            pattern IS the bf16 exp approximation (~2% multiplicative
            noise, zero-mean through softmax; loss rel-err ~3e-4).
  - PV emission lags the QK groups by 2 so exp latency (~1.2-1.5us) is
    hidden behind ~1.8us of PE work: stationary = expT chunk
    [128n, 128q], moving = v' [128n, 129]; PSUM accumulates z' | S.
  - ACT copies z'|S PSUM->SBUF right after the PV flush (frees the
    accumulation banks for the next attend with no DVE involvement);
    the DVE normalize/MSE math on those copies is deferred and
    interleaved into the NEXT attend's group loop, keeping the qc
    boundary free of serialized vector work:
      zcomp[qt] = z'c * 1/Sc   (compressed, via reciprocal + mul)
      acc[qt]  += sum((z'*invS - zcomp[qt])^2)  (two fused
                  scalar_tensor_tensor ops, accum_out row sums)
  - Compressed (NC=1024) and teacher (N=8192) attends interleave per
    q-chunk so the kT/vb DMA stream hides behind early compute.
"""

import numpy as np

import concourse.bass as bass
import concourse.mybir as mybir
import concourse.tile as tile
from concourse import bacc
from concourse.bass_utils import run_bass_kernel_spmd

F32 = mybir.dt.float32
BF16 = mybir.dt.bfloat16
I16 = mybir.dt.int16
AF = mybir.ActivationFunctionType
ALU = mybir.AluOpType

B, H, Q, N, NC, D = 1, 8, 1024, 8192, 1024, 128
N_CORES = 8
SCALE = 1.0 / float(np.sqrt(D))

QC = 256                   # q chunk width for the scores moving operand
N_QC = Q // QC             # 4
GT = 4                     # n-tiles per PSUM scores group (2 banks)
NT = N // 128              # 64 teacher n-tiles
NCT = NC // 128            # 8 compressed n-tiles
PV_LAG = 2                 # groups of PV emission lag behind QK

# Schraudolph-to-bf16 constants: exp(s*SCALE) ~= bf16_bits(rint(s*A16+B16)).
# HW DVE converts fp32->int16 with round-to-nearest (measured).
LN2 = float(np.log(2.0))
A16 = float(128.0 / LN2 * SCALE)
B16 = float(127 * 128 - 8)          # b_adj=8 minimizes softmax-weight bias


def teacher_on_dve(qc, gi):
    return gi % 2 == 1


def comp_on_dve(qc, gi):
    return gi == 1


def _emit(nc: bass.Bass, tc: tile.TileContext, qTh, kTh, vbh, ckTh, cvbh, out_dram):
    ctxs = []

    def pool(**kw):
        p = tc.tile_pool(**kw)
        ctxs.append(p)
        return p.__enter__()

    pconst = pool(name="pconst", bufs=1)
    pex = pool(name="pex", bufs=3)
    psmall = pool(name="psmall", bufs=8)
    psc = pool(name="psc", bufs=3, space="PSUM")
    pz = pool(name="pz", bufs=1, space="PSUM")

    # ---- persistent SBUF tensors ----
    kT = pconst.tile([128, NT, 128], BF16, tag="kT")        # [d, t, n]
    vb = pconst.tile([128, NT, 129], BF16, tag="vb")        # [p, t, d+1]
    qT = pconst.tile([128, Q], BF16, tag="qT")              # [d, q]
    ckT = pconst.tile([128, NCT, 128], BF16, tag="ckT")
    cvb = pconst.tile([128, NCT, 129], BF16, tag="cvb")
    zcomp = pconst.tile([128, Q // 128, 128], F32, tag="zcomp")  # [q, qt, d]
    accq = pconst.tile([128, Q // 128], F32, tag="accq")

    # Warm the ACT exp table immediately so the ~2.7us ACT_TABLE_LOAD is
    # off the first real exp's critical path.
    warm = psmall.tile([128, 1], F32, tag="warm")
    nc.gpsimd.memset(warm[:], 0.0)
    warm2 = psmall.tile([128, 1], F32, tag="warm2")
    nc.scalar.activation(warm2[:], warm[:], AF.Exp)

    # Warm the PE HAM clock gate during the input-DMA lead: ~3us of dummy
    # matmuls trips the activity monitor to K=8/8 (2.4 GHz) before the
    # first real matmul instead of ~8us into the compressed phase.
    wb = psmall.tile([128, 512], BF16, tag="wb")
    nc.gpsimd.memset(wb[:], 0.0)
    wps = psc.tile([128, GT, QC], F32, tag="sp")
    for _ in range(8):
        nc.tensor.matmul(wps[0:64, 0:2, :], wb[:, 0:64], wb[:],
                         start=True, stop=True)

    # ---- input DMAs (pre-transposed/swizzled bf16) spread over queues ----
    KCH = 4
    kt_per = NT // KCH

    def kchunk(c):
        return (kT[:, c * kt_per:(c + 1) * kt_per, :],
                kTh[:, c * kt_per * 128:(c + 1) * kt_per * 128]
                .rearrange("p (t n) -> p t n", t=kt_per))

    def vchunk(c):
        return (vb[:, c * kt_per:(c + 1) * kt_per, :],
                vbh[:, c * kt_per * 129:(c + 1) * kt_per * 129]
                .rearrange("p (t n) -> p t n", t=kt_per))

    nc.scalar.dma_start(out=ckT[:],
                        in_=ckTh[:, :].rearrange("p (t n) -> p t n", t=NCT))
    nc.sync.dma_start(out=qT[:, 0:QC], in_=qTh[:, 0:QC])
    nc.sync.dma_start(out=cvb[:],
                      in_=cvbh[:, :].rearrange("p (t n) -> p t n", t=NCT))
    # two parallel streams, ordered by first-use time; qT tail is not
    # needed until the second q-chunk so it goes last.
    for o, i in (kchunk(0), kchunk(1), vchunk(1), vchunk(3)):
        nc.sync.dma_start(out=o, in_=i)
    nc.sync.dma_start(out=qT[:, QC:Q], in_=qTh[:, QC:Q])
    for o, i in (kchunk(2), kchunk(3), vchunk(0), vchunk(2)):
        nc.gpsimd.dma_start(out=o, in_=i)

    # ---- attention + softmax-PV for one q-chunk of 256 ----
    def attend(keysT, vals, n_tiles, qc, on_dve, hooks=None):
        """Returns (za, zb) PSUM tiles [128, 129] = [z' | S] per q-half."""
        za = pz.tile([128, 129], F32, tag="za")
        zb = pz.tile([128, 129], F32, tag="zb")
        qs = qT[:, qc * QC:(qc + 1) * QC]

        def emit_pv(ex, t0, gn):
            for j in range(gn):
                t = t0 + j
                st = dict(start=(t == 0), stop=(t == n_tiles - 1))
                for c0, zp in ((0, za), (128, zb)):
                    nc.tensor.matmul(zp[:], ex[:, j, c0:c0 + 128],
                                     vals[:, t, :], **st)

        n_groups = n_tiles // GT
        pending = []
        for gi in range(n_groups):
            t0 = gi * GT
            sp = psc.tile([128, GT, QC], F32, tag="sp")
            for j in range(GT):
                nc.tensor.matmul(sp[:, j, :], keysT[:, t0 + j, :], qs,
                                 start=True, stop=True)
            if len(pending) >= PV_LAG:
                emit_pv(*pending.pop(0))
            ex = pex.tile([128, GT, QC], BF16, tag="ex")
            if on_dve(qc, gi):
                nc.vector.tensor_scalar(ex[:].bitcast(I16), sp[:], A16, B16,
                                        op0=ALU.mult, op1=ALU.add)
            else:
                nc.scalar.activation(ex[:], sp[:], AF.Exp, scale=SCALE)
            if hooks and gi in hooks:
                hooks[gi]()
            pending.append((ex, t0, GT))
        for p in pending:
            emit_pv(*p)
        return za, zb

    def act_drain(zp_pair):
        """ACT copies z'|S PSUM->SBUF, freeing the accumulation banks."""
        out = []
        for zp in zp_pair:
            zs = psmall.tile([128, 129], F32, tag="zs")
            nc.scalar.copy(zs[:], zp[:])
            out.append(zs)
        return out

    def zcomp_normalize(czs, qt):
        def run():
            inv = psmall.tile([128, 1], F32, tag="cinv")
            nc.vector.reciprocal(inv[:], czs[:, 128:129])
            nc.vector.tensor_scalar_mul(zcomp[:, qt, :], czs[:, 0:128], inv[:])
        return run

    def mse(zs, qt):
        def run():
            inv = psmall.tile([128, 1], F32, tag="inv")
            nc.vector.reciprocal(inv[:], zs[:, 128:129])
            d = psmall.tile([128, 128], F32, tag="d")
            nc.vector.scalar_tensor_tensor(d[:], zs[:, 0:128], inv[:],
                                           zcomp[:, qt, :],
                                           op0=ALU.mult, op1=ALU.subtract)
            d2 = psmall.tile([128, 128], F32, tag="d2")
            nc.vector.scalar_tensor_tensor(d2[:], d[:], 1.0, d[:],
                                           op0=ALU.mult, op1=ALU.mult,
                                           accum_out=accq[:, qt:qt + 1])
        return run

    # Interleaved phases per q-chunk; the DVE normalize/MSE of earlier
    # attends runs inside the next teacher attend's group loop.
    prev_mse = []
    for qc in range(N_QC):
        za, zb = attend(ckT, cvb, NCT, qc, comp_on_dve)
        czs = act_drain((za, zb))

        hooks = {2: zcomp_normalize(czs[0], qc * 2),
                 3: zcomp_normalize(czs[1], qc * 2 + 1)}
        for i, fn in enumerate(prev_mse):
            hooks[5 + 3 * i] = fn
        za, zb = attend(kT, vb, NT, qc, teacher_on_dve, hooks)
        zs = act_drain((za, zb))
        prev_mse = [mse(zs[0], qc * 2), mse(zs[1], qc * 2 + 1)]

    for fn in prev_mse:
        fn()

    nc.sync.dma_start(out=out_dram[:], in_=accq[:])

    for p in reversed(ctxs):
        p.__exit__(None, None, None)


_NC_CACHE = None


def build_nc():
    global _NC_CACHE
    if _NC_CACHE is not None:
        return _NC_CACHE
    nc = bacc.Bacc()
    qTh = nc.declare_dram_parameter("qT", [128, Q], BF16, isOutput=False)
    kTh = nc.declare_dram_parameter("kT", [128, N], BF16, isOutput=False)
    vbh = nc.declare_dram_parameter("vb", [128, NT * 129], BF16, isOutput=False)
    ckTh = nc.declare_dram_parameter("ckT", [128, NC], BF16, isOutput=False)
    cvbh = nc.declare_dram_parameter("cvb", [128, NCT * 129], BF16, isOutput=False)
    out = nc.declare_dram_parameter("loss_sums", [128, Q // 128], F32, isOutput=True)
    with tile.TileContext(nc) as tc:
        _emit(nc, tc, qTh, kTh, vbh, ckTh, cvbh, out)
    nc.compile()
    _NC_CACHE = nc
    return nc


NPBF16 = mybir.dt.np(BF16)


def _prep_head(qh, kh, vh, ckh, cvh):
    """Host-side shard prep: transpose/swizzle/cast one head's operands."""
    def swizzle_v(v):              # [n, d] -> [128p, t, d+1] with ones col
        t = v.shape[0] // 128
        vs = v.reshape(t, 128, D).transpose(1, 0, 2)
        out = np.empty((128, t, D + 1), dtype=NPBF16)
        out[:, :, 0:D] = vs.astype(NPBF16)
        out[:, :, D] = np.asarray(1.0, dtype=NPBF16)
        return out.reshape(128, t * (D + 1))

    return {
        "qT": np.ascontiguousarray(qh.T).astype(NPBF16),
        "kT": np.ascontiguousarray(kh.T).astype(NPBF16),
        "vb": swizzle_v(vh),
        "ckT": np.ascontiguousarray(ckh.T).astype(NPBF16),
        "cvb": swizzle_v(cvh),
    }


def make_in_maps(queries, keys, values, c_keys, c_values):
    in_maps = []
    for h in range(N_CORES):
        in_maps.append(_prep_head(
            np.asarray(queries[0, h], dtype=np.float32),
            np.asarray(keys[0, h], dtype=np.float32),
            np.asarray(values[0, h], dtype=np.float32),
            np.asarray(c_keys[0, h], dtype=np.float32),
            np.asarray(c_values[0, h], dtype=np.float32),
        ))
    return in_maps


def run_cores(in_maps, trace=False, **kw):
    nc = build_nc()
    return run_bass_kernel_spmd(nc, in_maps, list(range(N_CORES)),
                                trace=trace, **kw)


def kernel(queries, keys, values, c_keys, c_values):
    res = run_cores(make_in_maps(queries, keys, values, c_keys, c_values))
    total = sum(float(r["loss_sums"].astype(np.float64).sum())
                for r in res.results)
    loss = total / float(B * H * Q * D)
    return np.asarray(loss, dtype=np.float32)


# revision 12
# speedup vs baseline: 1.0925x; 1.0899x over previous
"""Distillation-trainer loss kernel for Trainium2 (8 NeuronCores).

Computes  loss = mean((attn(q,k,v) - attn(q,ck,cv))**2)  for
q:[1,8,1024,128], k/v:[1,8,8192,128], ck/cv:[1,8,1024,128] fp32.

Sharding: one kv-head per core (h axis, 8 heads / 8 cores). Each core
computes its head's squared-error partial sums; the host adds the 8
partials and divides by the element count (the "all-reduce" of the
scalar loss).

Host-side prep (part of sharding): per head, ship bf16 operands in the
exact SBUF layouts the PE needs — kT/qT/ckT pre-transposed to [d, n],
v/cv pre-swizzled to [128p, t, d] with a ones column appended (the
denominator trick). This removes all on-device transposes/casts and
halves DMA bytes. Input DMAs are spread across engine queues so the
kT and vb streams transfer in parallel.

Per-core algorithm (head h), per 256-wide q-chunk:
  - scoresT[n, q] = kT-tile.T @ qT-chunk on PE in bf16 (fp32 PSUM).
    Scores grouped 4 n-tiles (2 PSUM banks) x 3 buffers so TWO exp
    engines run concurrently on different groups:
      ACT:  expT = Exp(scoresT * 1/sqrt(d)) -> bf16 (even groups).
      DVE:  Schraudolph in bf16 (odd groups): i16 = rint(s*A16 + B16)
            written through a bitcast into the bf16 tile; the int16 bit
            pattern IS the bf16 exp approximation (~2% multiplicative
            noise, zero-mean through softmax; loss rel-err ~3e-4).
  - PV emission lags the QK groups by 2 so exp latency (~1.2-1.5us) is
    hidden behind ~1.8us of PE work: stationary = expT chunk
    [128n, 128q], moving = v' [128n, 129]; PSUM accumulates z' | S.
  - ACT copies z'|S PSUM->SBUF right after the PV flush (frees the
    accumulation banks for the next attend with no DVE involvement);
    the DVE normalize/MSE math on those copies is deferred and
    interleaved into the NEXT attend's group loop, keeping the qc
    boundary free of serialized vector work:
      zcomp[qt] = z'c * 1/Sc   (compressed, via reciprocal + mul)
      acc[qt]  += sum((z'*invS - zcomp[qt])^2)  (two fused
                  scalar_tensor_tensor ops, accum_out row sums)
  - Compressed (NC=1024) and teacher (N=8192) attends interleave per
    q-chunk so the kT/vb DMA stream hides behind early compute.
"""

import numpy as np

import concourse.bass as bass
import concourse.mybir as mybir
import concourse.tile as tile
from concourse import bacc
from concourse.bass_utils import run_bass_kernel_spmd

F32 = mybir.dt.float32
BF16 = mybir.dt.bfloat16
I16 = mybir.dt.int16
AF = mybir.ActivationFunctionType
ALU = mybir.AluOpType

B, H, Q, N, NC, D = 1, 8, 1024, 8192, 1024, 128
N_CORES = 8
SCALE = 1.0 / float(np.sqrt(D))

QC = 256                   # q chunk width for the scores moving operand
N_QC = Q // QC             # 4
GT = 4                     # n-tiles per PSUM scores group (2 banks)
NT = N // 128              # 64 teacher n-tiles
NCT = NC // 128            # 8 compressed n-tiles
PV_LAG = 2                 # groups of PV emission lag behind QK

# Schraudolph-to-bf16 constants: exp(s*SCALE) ~= bf16_bits(rint(s*A16+B16)).
# HW DVE converts fp32->int16 with round-to-nearest (measured).
LN2 = float(np.log(2.0))
A16 = float(128.0 / LN2 * SCALE)
B16 = float(127 * 128 - 8)          # b_adj=8 minimizes softmax-weight bias


def teacher_on_dve(qc, gi):
    return gi % 2 == 1


def comp_on_dve(qc, gi):
    return gi == 1


def _emit(nc: bass.Bass, tc: tile.TileContext, qTh, kTh, vbh, ckTh, cvbh, out_dram):
    ctxs = []

    def pool(**kw):
        p = tc.tile_pool(**kw)
        ctxs.append(p)
        return p.__enter__()

    pconst = pool(name="pconst", bufs=1)
    pex = pool(name="pex", bufs=3)
    psmall = pool(name="psmall", bufs=8)
    psc = pool(name="psc", bufs=3, space="PSUM")
    pz = pool(name="pz", bufs=1, space="PSUM")

    # ---- persistent SBUF tensors ----
    kT = pconst.tile([128, NT, 128], BF16, tag="kT")        # [d, t, n]
    vb = pconst.tile([128, NT, 129], BF16, tag="vb")        # [p, t, d+1]
    qT = pconst.tile([128, Q], BF16, tag="qT")              # [d, q]
    ckT = pconst.tile([128, NCT, 128], BF16, tag="ckT")
    cvb = pconst.tile([128, NCT, 129], BF16, tag="cvb")
    zcomp = pconst.tile([128, Q // 128, 128], F32, tag="zcomp")  # [q, qt, d]
    accq = pconst.tile([128, Q // 128], F32, tag="accq")

    # Warm the ACT exp table immediately so the ~2.7us ACT_TABLE_LOAD is
    # off the first real exp's critical path.
    warm = psmall.tile([128, 1], F32, tag="warm")
    nc.gpsimd.memset(warm[:], 0.0)
    warm2 = psmall.tile([128, 1], F32, tag="warm2")
    nc.scalar.activation(warm2[:], warm[:], AF.Exp)

    # Warm the PE HAM clock gate during the input-DMA lead: ~3us of dummy
    # matmuls trips the activity monitor to K=8/8 (2.4 GHz) before the
    # first real matmul instead of ~8us into the compressed phase.
    wb = psmall.tile([128, 512], BF16, tag="wb")
    nc.gpsimd.memset(wb[:], 0.0)
    wps = psc.tile([128, GT, QC], F32, tag="sp")
    for _ in range(8):
        nc.tensor.matmul(wps[0:64, 0:2, :], wb[:, 0:64], wb[:],
                         start=True, stop=True)

    # ---- input DMAs (pre-transposed/swizzled bf16) ----
    # The 5.1MB stream takes ~15us of wire time; one queue, emitted in
    # just-in-time consumption order, acts as a priority schedule. The
    # DMA_DIRECT2D "durations" are descriptor-issue only; actual packets
    # trickle until ~22us, so order is everything here.
    KCH = 8
    kt_per = NT // KCH

    def kchunk(c):
        return (kT[:, c * kt_per:(c + 1) * kt_per, :],
                kTh[:, c * kt_per * 128:(c + 1) * kt_per * 128]
                .rearrange("p (t n) -> p t n", t=kt_per))

    def vchunk(c):
        return (vb[:, c * kt_per:(c + 1) * kt_per, :],
                vbh[:, c * kt_per * 129:(c + 1) * kt_per * 129]
                .rearrange("p (t n) -> p t n", t=kt_per))

    nc.sync.dma_start(out=ckT[:],
                      in_=ckTh[:, :].rearrange("p (t n) -> p t n", t=NCT))
    nc.sync.dma_start(out=qT[:, 0:QC], in_=qTh[:, 0:QC])
    nc.sync.dma_start(out=cvb[:],
                      in_=cvbh[:, :].rearrange("p (t n) -> p t n", t=NCT))
    stream = [kchunk(0), kchunk(1), vchunk(0), kchunk(2), vchunk(1),
              kchunk(3), vchunk(2), kchunk(4), vchunk(3), kchunk(5),
              vchunk(4), kchunk(6), vchunk(5), kchunk(7), vchunk(6),
              vchunk(7)]
    for o, i in stream:
        nc.sync.dma_start(out=o, in_=i)
    nc.sync.dma_start(out=qT[:, QC:Q], in_=qTh[:, QC:Q])

    # ---- attention + softmax-PV for one q-chunk of 256 ----
    def attend(keysT, vals, n_tiles, qc, on_dve, hooks=None):
        """Returns (za, zb) PSUM tiles [128, 129] = [z' | S] per q-half."""
        za = pz.tile([128, 129], F32, tag="za")
        zb = pz.tile([128, 129], F32, tag="zb")
        qs = qT[:, qc * QC:(qc + 1) * QC]

        def emit_pv(ex, t0, gn):
            for j in range(gn):
                t = t0 + j
                st = dict(start=(t == 0), stop=(t == n_tiles - 1))
                for c0, zp in ((0, za), (128, zb)):
                    nc.tensor.matmul(zp[:], ex[:, j, c0:c0 + 128],
                                     vals[:, t, :], **st)

        n_groups = n_tiles // GT
        pending = []
        for gi in range(n_groups):
            t0 = gi * GT
            sp = psc.tile([128, GT, QC], F32, tag="sp")
            for j in range(GT):
                nc.tensor.matmul(sp[:, j, :], keysT[:, t0 + j, :], qs,
                                 start=True, stop=True)
            if len(pending) >= PV_LAG:
                emit_pv(*pending.pop(0))
            ex = pex.tile([128, GT, QC], BF16, tag="ex")
            if on_dve(qc, gi):
                nc.vector.tensor_scalar(ex[:].bitcast(I16), sp[:], A16, B16,
                                        op0=ALU.mult, op1=ALU.add)
            else:
                nc.scalar.activation(ex[:], sp[:], AF.Exp, scale=SCALE)
            if hooks and gi in hooks:
                hooks[gi]()
            pending.append((ex, t0, GT))
        for p in pending:
            emit_pv(*p)
        return za, zb

    def act_drain(zp_pair):
        """ACT copies z'|S PSUM->SBUF, freeing the accumulation banks."""
        out = []
        for zp in zp_pair:
            zs = psmall.tile([128, 129], F32, tag="zs")
            nc.scalar.copy(zs[:], zp[:])
            out.append(zs)
        return out

    def zcomp_normalize(czs, qt):
        def run():
            inv = psmall.tile([128, 1], F32, tag="cinv")
            nc.vector.reciprocal(inv[:], czs[:, 128:129])
            nc.vector.tensor_scalar_mul(zcomp[:, qt, :], czs[:, 0:128], inv[:])
        return run

    def mse(zs, qt):
        def run():
            inv = psmall.tile([128, 1], F32, tag="inv")
            nc.vector.reciprocal(inv[:], zs[:, 128:129])
            d = psmall.tile([128, 128], F32, tag="d")
            nc.vector.scalar_tensor_tensor(d[:], zs[:, 0:128], inv[:],
                                           zcomp[:, qt, :],
                                           op0=ALU.mult, op1=ALU.subtract)
            d2 = psmall.tile([128, 128], F32, tag="d2")
            nc.vector.scalar_tensor_tensor(d2[:], d[:], 1.0, d[:],
                                           op0=ALU.mult, op1=ALU.mult,
                                           accum_out=accq[:, qt:qt + 1])
        return run

    # Interleaved phases per q-chunk; the DVE normalize/MSE of earlier
    # attends runs inside the next teacher attend's group loop.
    prev_mse = []
    for qc in range(N_QC):
        za, zb = attend(ckT, cvb, NCT, qc, comp_on_dve)
        czs = act_drain((za, zb))

        hooks = {2: zcomp_normalize(czs[0], qc * 2),
                 3: zcomp_normalize(czs[1], qc * 2 + 1)}
        for i, fn in enumerate(prev_mse):
            hooks[5 + 3 * i] = fn
        za, zb = attend(kT, vb, NT, qc, teacher_on_dve, hooks)
        zs = act_drain((za, zb))
        prev_mse = [mse(zs[0], qc * 2), mse(zs[1], qc * 2 + 1)]

    for fn in prev_mse:
        fn()

    nc.sync.dma_start(out=out_dram[:], in_=accq[:])

    for p in reversed(ctxs):
        p.__exit__(None, None, None)


_NC_CACHE = None


def build_nc():
    global _NC_CACHE
    if _NC_CACHE is not None:
        return _NC_CACHE
    nc = bacc.Bacc()
    qTh = nc.declare_dram_parameter("qT", [128, Q], BF16, isOutput=False)
    kTh = nc.declare_dram_parameter("kT", [128, N], BF16, isOutput=False)
    vbh = nc.declare_dram_parameter("vb", [128, NT * 129], BF16, isOutput=False)
    ckTh = nc.declare_dram_parameter("ckT", [128, NC], BF16, isOutput=False)
    cvbh = nc.declare_dram_parameter("cvb", [128, NCT * 129], BF16, isOutput=False)
    out = nc.declare_dram_parameter("loss_sums", [128, Q // 128], F32, isOutput=True)
    with tile.TileContext(nc) as tc:
        _emit(nc, tc, qTh, kTh, vbh, ckTh, cvbh, out)
    nc.compile()
    _NC_CACHE = nc
    return nc


NPBF16 = mybir.dt.np(BF16)


def _prep_head(qh, kh, vh, ckh, cvh):
    """Host-side shard prep: transpose/swizzle/cast one head's operands."""
    def swizzle_v(v):              # [n, d] -> [128p, t, d+1] with ones col
        t = v.shape[0] // 128
        vs = v.reshape(t, 128, D).transpose(1, 0, 2)
        out = np.empty((128, t, D + 1), dtype=NPBF16)
        out[:, :, 0:D] = vs.astype(NPBF16)
        out[:, :, D] = np.asarray(1.0, dtype=NPBF16)
        return out.reshape(128, t * (D + 1))

    return {
        "qT": np.ascontiguousarray(qh.T).astype(NPBF16),
        "kT": np.ascontiguousarray(kh.T).astype(NPBF16),
        "vb": swizzle_v(vh),
        "ckT": np.ascontiguousarray(ckh.T).astype(NPBF16),
        "cvb": swizzle_v(cvh),
    }


def make_in_maps(queries, keys, values, c_keys, c_values):
    in_maps = []
    for h in range(N_CORES):
        in_maps.append(_prep_head(
            np.asarray(queries[0, h], dtype=np.float32),
            np.asarray(keys[0, h], dtype=np.float32),
            np.asarray(values[0, h], dtype=np.float32),
            np.asarray(c_keys[0, h], dtype=np.float32),
            np.asarray(c_values[0, h], dtype=np.float32),
        ))
    return in_maps


def run_cores(in_maps, trace=False, **kw):
    nc = build_nc()
    return run_bass_kernel_spmd(nc, in_maps, list(range(N_CORES)),
                                trace=trace, **kw)


def kernel(queries, keys, values, c_keys, c_values):
    res = run_cores(make_in_maps(queries, keys, values, c_keys, c_values))
    total = sum(float(r["loss_sums"].astype(np.float64).sum())
                for r in res.results)
    loss = total / float(B * H * Q * D)
    return np.asarray(loss, dtype=np.float32)
